# revision 24
# baseline (speedup 1.0000x reference)
"""Distributed Trainium2 Bass kernel for causal multi-head attention w/ RoPE.

Problem shapes (hardcoded): B=2, S=2048, D=1024, H=16, HD=64.
Sharding: tensor-parallel over heads — each of 8 cores owns 2 heads
(column slice of wq/wk/wv, row slice of wo). Each core emits its partial
x @ woT contribution; the host sums the 8 partials (the "all-reduce").

Per-core math (all matmuls bf16 on the PE, fp32 PSUM accumulation):
  - q,k,v projections from a host-transposed xT [D, B*S].
  - RoPE: rot(q) = q*cos + (P@q)*sin with P a signed block-swap applied
    by one PE matmul; DVE elementwise ops finish the rotation.
  - scores computed transposed: sT[sk, sq] = k_h^T q_h (K=64), causal
    tiles only; exp on ScalarE with the 1/sqrt(HD) scale folded in.
  - causal mask applied ON the PE (identity @ maskT accumulated into the
    scores PSUM).
  - SLAB-MAJOR schedule: attention runs one 512-wide sq slab at a time
    (for kt <= slab end: scores+mask, exp, PV for both heads,
    interleaved).  The slab's two PV accumulators [65,512] live only for
    the slab => PSUM: 4 banks of score ring + 2 banks po + 2 banks for
    filler (projections / rope / v-transpose / wo).
  - PV uses v' = [v | 1] so the softmax denominator falls out of the
    matmul as row 64; normalization = DVE copies + gpsimd
    partition-broadcast + DVE reciprocal + multiply.
  - the PE never idles: between attention rounds the emitter injects
    dependency-free filler (next batch's projections, rope, v', and the
    wo matmuls of ALREADY-normalized slabs).  Each slab's wo becomes
    filler for the next slab, so the drain tail is one slab's wo.
  - input DMA: fine-grained chunks spread over 4 engine queues (per
    queue the transfers serialize at ~60GB/s, so parallelism across
    queues is what sets arrival rate).
"""

import sys

sys.path.insert(0, "/opt/trn_rl_repo")

import numpy as np
import ml_dtypes

B, S, D, H = 2, 2048, 1024, 16
HD = D // H  # 64
NC = 8
HPC = H // NC  # heads per core = 2
HDC = HPC * HD  # head dims per core = 128
TOK = B * S  # 4096
BF16 = ml_dtypes.bfloat16

_COMPILED = {}


def _build_program():
    import concourse.bass as bass
    import concourse.mybir as mybir
    import concourse.bacc as bacc
    from concourse import tile

    f32 = mybir.dt.float32
    bf16 = mybir.dt.bfloat16
    MULT = mybir.AluOpType.mult
    ADD = mybir.AluOpType.add
    EXP = mybir.ActivationFunctionType.Exp
    KT = D // 128  # 8 contraction tiles for projections
    NTB = TOK // 512  # 8 tok blocks of 512
    NVT = TOK // 128  # 32 tok tiles of 128 (v' tiles)
    VW = HD + 1  # 65: v plus ones column

    nc = bacc.Bacc("TRN2", target_bir_lowering=False, debug=False, num_devices=NC)

    # xT relaid on host: row tb*128+p, col k*512+c = xT[k*128+p, tb*512+c]
    xT_d = nc.dram_tensor("xT", [NTB * 128, KT * 512], bf16,
                          kind="ExternalInput").ap()
    # weights relaid: row p, col k*HDC+c = wT[k*128+p, c]
    wqT_d = nc.dram_tensor("wqT", [128, KT * HDC], bf16, kind="ExternalInput").ap()
    wkT_d = nc.dram_tensor("wkT", [128, KT * HDC], bf16, kind="ExternalInput").ap()
    wvT_d = nc.dram_tensor("wvT", [128, KT * HDC], bf16, kind="ExternalInput").ap()
    woT_d = nc.dram_tensor("woT", [HDC, D], bf16, kind="ExternalInput").ap()
    PT_d = nc.dram_tensor("PT", [HDC, HDC], bf16, kind="ExternalInput").ap()
    cos_d = nc.dram_tensor("cosx", [HDC, TOK], bf16, kind="ExternalInput").ap()
    sin_d = nc.dram_tensor("sinx", [HDC, TOK], bf16, kind="ExternalInput").ap()
    tri_d = nc.dram_tensor("tri", [128, 128], bf16, kind="ExternalInput").ap()
    id_d = nc.dram_tensor("ident", [128, 128], bf16, kind="ExternalInput").ap()
    out_d = nc.dram_tensor("out", [D, TOK], bf16, kind="ExternalOutput").ap()

    with tile.TileContext(nc) as tc:
        with (
            tc.tile_pool(name="big", bufs=1) as big,
            tc.tile_pool(name="work", bufs=3) as work,
            tc.tile_pool(name="etp", bufs=8) as etp,
            tc.tile_pool(name="nrm", bufs=3) as nrm,
            tc.tile_pool(name="scps", bufs=4, space="PSUM") as scps,
            tc.tile_pool(name="povp", bufs=2, space="PSUM") as povp,
            tc.tile_pool(name="mscp", bufs=2, space="PSUM") as mscp,
        ):
            # ---- DMA: fine chunks over 4 queues, first-needed first ----
            wq = big.tile([128, KT * HDC], bf16, tag="wq")
            wk = big.tile([128, KT * HDC], bf16, tag="wk")
            wv = big.tile([128, KT * HDC], bf16, tag="wv")
            xT = big.tile([128, KT, TOK], bf16, tag="xT")
            cosx = big.tile([128, TOK], bf16, tag="cosx")
            sinx = big.tile([128, TOK], bf16, tag="sinx")
            tri = big.tile([128, 128], bf16, tag="tri")
            ident = big.tile([128, 128], bf16, tag="ident")
            PT = big.tile([128, 128], bf16, tag="PT")
            wo = big.tile([128, D], bf16, tag="wo")

            def x_dma(tb, k0, k1, eng):
                eng.dma_start(
                    xT[:, k0:k1, tb * 512 : (tb + 1) * 512],
                    xT_d[tb * 128 : (tb + 1) * 128, k0 * 512 : k1 * 512])

            # queue scripts.  Only SP(sync)/Pool(gpsimd)/Act(scalar) can
            # initiate DMA.  A big transfer BLOCKS its issuing queue, so
            # the scalar queue gets only small, early transfers (it must
            # be free for exp once attention starts ~18us in); sync and
            # gpsimd carry the x stream (the 16 HW DMA engines give each
            # queue ~100GB/s).
            nc.sync.dma_start(wq[:], wqT_d[:, :])
            x_dma(0, 0, 2, nc.gpsimd)
            nc.scalar.dma_start(wk[:], wkT_d[:, :])
            x_dma(0, 2, 4, nc.sync)
            x_dma(0, 4, 6, nc.gpsimd)
            nc.scalar.dma_start(wv[:], wvT_d[:, :])
            x_dma(0, 6, 8, nc.sync)
            nc.scalar.dma_start(tri[:], tri_d[:, :])
            nc.scalar.dma_start(ident[:], id_d[:, :])
            nc.scalar.dma_start(PT[:], PT_d[:, :])
            nc.gpsimd.dma_start(cosx[:, 0:1024], cos_d[:, 0:1024])
            nc.sync.dma_start(sinx[:, 0:1024], sin_d[:, 0:1024])
            x_dma(1, 0, 4, nc.gpsimd)
            x_dma(1, 4, 8, nc.sync)
            nc.scalar.dma_start(cosx[:, 1024:2048], cos_d[:, 1024:2048])
            nc.scalar.dma_start(sinx[:, 1024:2048], sin_d[:, 1024:2048])
            x_dma(2, 0, 4, nc.sync)
            x_dma(2, 4, 8, nc.gpsimd)
            nc.scalar.dma_start(wo[:], woT_d[:, :])
            x_dma(3, 0, 4, nc.gpsimd)
            x_dma(3, 4, 8, nc.sync)
            nc.gpsimd.dma_start(cosx[:, 2048:4096], cos_d[:, 2048:4096])
            nc.sync.dma_start(sinx[:, 2048:4096], sin_d[:, 2048:4096])
            for tb in range(4, NTB):
                qa, qb = ((nc.gpsimd, nc.sync),
                          (nc.sync, nc.gpsimd))[tb % 2]
                x_dma(tb, 0, 4, qa)
                x_dma(tb, 4, 8, qb)

            q_sb = big.tile([128, TOK], bf16, tag="q")
            k_sb = big.tile([128, TOK], bf16, tag="k")
            v_sb = big.tile([128, TOK], bf16, tag="v")
            rotq = big.tile([128, TOK], bf16, tag="rotq")
            rotk = big.tile([128, TOK], bf16, tag="rotk")
            vp = big.tile([128, NVT * 2 * VW], bf16, tag="vp")
            oh1 = big.tile([64, S], bf16, tag="oh1")
            outT = {}
            for b in range(B):
                outT[b] = big.tile([128, S], bf16, tag=f"outT{b}",
                                   name=f"outT{b}")

            def vp_head(kt, h):
                base = kt * 2 * VW + h * VW
                return vp[:, base : base + VW]

            # ---------- filler units (each = one mscp psum lifetime) ----
            copy_flip = [0]

            def u_proj(tb, di):
                """one projection dst (q/k/v) for token block tb"""
                dst, w_sb = ((q_sb, wq), (k_sb, wk), (v_sb, wv))[di]
                ps = mscp.tile([128, 512], f32, tag="pj", name=f"pj{tb}{di}")
                for k in range(KT):
                    nc.tensor.matmul(
                        ps[:], w_sb[:, k * HDC : (k + 1) * HDC],
                        xT[:, k, tb * 512 : (tb + 1) * 512],
                        start=(k == 0), stop=(k == KT - 1))
                copy_flip[0] ^= 1
                eng = nc.scalar if copy_flip[0] else nc.vector
                if eng is nc.scalar:
                    nc.scalar.copy(dst[:, tb * 512 : (tb + 1) * 512], ps[:])
                else:
                    nc.vector.tensor_copy(dst[:, tb * 512 : (tb + 1) * 512],
                                          ps[:])

            def u_rope(tb, qi):
                """rope for q (qi=0) or k (qi=1) on token block tb"""
                src, dst = ((q_sb, rotq), (k_sb, rotk))[qi]
                blk = slice(tb * 512, (tb + 1) * 512)
                pss = mscp.tile([128, 512], f32, tag="pj", name=f"pr{tb}{qi}")
                nc.tensor.matmul(pss[:], PT[:], src[:, blk],
                                 start=True, stop=True)
                t1 = work.tile([128, 512], bf16, tag="ropet1")
                nc.vector.tensor_tensor(t1[:], src[:, blk], cosx[:, blk], MULT)
                t2 = work.tile([128, 512], bf16, tag="ropet2")
                nc.vector.tensor_tensor(t2[:], pss[:], sinx[:, blk], MULT)
                nc.vector.tensor_tensor(dst[:, blk], t1[:], t2[:], ADD)

            def u_vp(tb, half):
                """v' tiles for 2 of tb's 4 token tiles (half=0/1)"""
                for kt in (tb * 4 + 2 * half, tb * 4 + 2 * half + 1):
                    pst = mscp.tile([128, 128], bf16, tag="pj",
                                    name=f"pv{kt}")
                    nc.tensor.transpose(pst[:],
                                        v_sb[:, kt * 128 : (kt + 1) * 128],
                                        ident[:])
                    for h in range(HPC):
                        base = kt * 2 * VW + h * VW
                        nc.vector.tensor_copy(vp[:, base : base + HD],
                                              pst[:, h * HD : (h + 1) * HD])
                        nc.gpsimd.memset(vp[:, base + HD : base + VW], 1.0)

            def u_wo(bw, t4, o, drain=False):
                """one wo output tile: matmul + copy + dma"""
                psw = mscp.tile([128, 512], f32, tag="pj",
                                name=f"pw{bw}{t4}{o}")
                nc.tensor.matmul(
                    psw[:], wo[:, o * 128 : (o + 1) * 128],
                    outT[bw][:, t4 * 512 : (t4 + 1) * 512],
                    start=True, stop=True)
                wout = work.tile([128, 512], bf16, tag="wout")
                if drain and o % 2:
                    # only the drain tail may use ScalarE: during attention
                    # it must be exp-only (exp paces the whole b1 era)
                    nc.scalar.copy(wout[:], psw[:])
                else:
                    nc.vector.tensor_copy(wout[:], psw[:])
                (nc.sync, nc.gpsimd)[o % 2].dma_start(
                    out_d[o * 128 : (o + 1) * 128,
                          bw * S + t4 * 512 : bw * S + (t4 + 1) * 512],
                    wout[:])

            # unit registry: every filler unit is keyed; slabs force-emit
            # their prerequisites (idempotent), extra units pump as filler
            done = set()

            def emit_unit(key):
                if key in done:
                    return
                done.add(key)
                kind = key[0]
                if kind == "proj":
                    u_proj(key[1], key[2])
                elif kind == "rope":
                    u_rope(key[1], key[2])
                elif kind == "vp":
                    u_vp(key[1], key[2])
                elif kind == "wo":
                    u_wo(key[1], key[2], key[3])

            def tb_keys(tb):
                return ([("proj", tb, di) for di in range(3)]
                        + [("rope", tb, 0), ("rope", tb, 1)]
                        + [("vp", tb, 0), ("vp", tb, 1)])

            fillers = []

            def pump(n):
                while n > 0 and fillers:
                    key = fillers.pop(0)
                    if key in done:
                        continue
                    emit_unit(key)
                    n -= 1

            # ---------------- attention slab ---------------------------
            def slab(b, g, i, per_round=1):
                """sq slab [c0, c0+512) of batch b; kt = 0..n_kt-1.

                Per round: S(h1),E(h1),PV(h1,kt-1),S(h0),E(h0),PV(h0,kt-1)
                with `per_round` filler units pumped each round."""
                c0 = 1024 * g + 512 * i
                n_kt = (c0 + 512) // 128
                po = {}
                for h in (1, 0):
                    po[h] = povp.tile([VW, 512], f32, tag="po",
                                      name=f"po{b}{g}{i}{h}")
                sc = {}
                et = {}

                def norm(h):
                    """normalize po[h] -> outT[b] rows h*64..h*64+64.
                    po is read in place (ring 2 = a full slab of slack)."""
                    d0 = nrm.tile([1, 512], f32, tag="d0")
                    nc.vector.tensor_copy(d0[0:1, :], po[h][HD : HD + 1, :])
                    db = nrm.tile([64, 512], f32, tag="db")
                    nc.gpsimd.partition_broadcast(db[:, :], d0[0:1, :])
                    rb_sb = nrm.tile([64, 512], f32, tag="rb")
                    nc.vector.reciprocal_approx_fast(rb_sb[:, :], db[:, :])
                    ocols = slice(c0, c0 + 512)
                    if h == 0:
                        nc.vector.tensor_tensor(outT[b][0:HD, ocols],
                                                po[h][0:HD, :], rb_sb[:, :],
                                                MULT)
                    else:
                        nc.vector.tensor_tensor(oh1[:, ocols], po[h][0:HD, :],
                                                rb_sb[:, :], MULT)
                        nc.sync.dma_start(outT[b][HD : 2 * HD, ocols],
                                          oh1[:, ocols])

                def emit_S(h, kt):
                    hsl = slice(h * HD, (h + 1) * HD)
                    w0 = 128 * kt
                    s0 = max(w0, c0)
                    t = scps.tile([128, 512], f32, tag="sc",
                                  name=f"sc{b}{g}{i}{h}{kt}")
                    sc[(h, kt)] = (t, s0)
                    diag = w0 >= c0
                    nc.tensor.matmul(
                        t[:, s0 - c0 : 512],
                        rotk[hsl, b * S + w0 : b * S + w0 + 128],
                        rotq[hsl, b * S + s0 : b * S + c0 + 512],
                        start=True, stop=not diag,
                    )
                    if diag:
                        nc.tensor.matmul(
                            t[:, w0 - c0 : w0 - c0 + 128],
                            ident[:], tri[:],
                            start=False, stop=True,
                        )

                def emit_E(h, kt):
                    t, s0 = sc.pop((h, kt))
                    tt = etp.tile([128, 512], bf16, tag="expT",
                                  name=f"et{b}{g}{i}{h}{kt}")
                    et[(h, kt)] = (tt, s0)
                    nc.scalar.activation(
                        tt[:, s0 - c0 : 512], t[:, s0 - c0 : 512],
                        EXP, scale=0.125)

                def emit_PV(h, kt):
                    tt, s0 = et.pop((h, kt))
                    nc.tensor.matmul(
                        po[h][:, s0 - c0 : 512],
                        vp_head(b * (NVT // B) + kt, h),
                        tt[:, s0 - c0 : 512],
                        start=(kt == 0), stop=(kt == n_kt - 1),
                    )

                emit_S(1, 0)
                emit_E(1, 0)
                emit_S(0, 0)
                emit_E(0, 0)
                for kt in range(1, n_kt):
                    emit_S(1, kt)
                    emit_E(1, kt)
                    emit_PV(1, kt - 1)
                    pump(per_round - 1)
                    emit_S(0, kt)
                    emit_E(0, kt)
                    emit_PV(0, kt - 1)
                    pump(1)
                emit_PV(1, n_kt - 1)
                norm(1)
                emit_PV(0, n_kt - 1)
                norm(0)

            # ---------------- global schedule ---------------------------
            # slab order: b0 causal order, then b1 with the smallest slab
            # (4 rounds) LAST so the drain tail is only its 8 wo tiles.
            # Before each slab, force-emit its prerequisite tb units
            # (idempotent); queued copies pump as filler inside rounds.
            order = [(0, 0, 0), (0, 0, 1), (0, 1, 0), (0, 1, 1),
                     (1, 0, 1), (1, 1, 0), (1, 1, 1), (1, 0, 0)]
            # seed future-tb units into the filler queue ahead of need
            prefeed = {
                (0, 0, 0): [1],  # during slab 000, start tb1
                (0, 0, 1): [2],
                (0, 1, 0): [3],
                (0, 1, 1): [4, 5],
                (1, 0, 1): [6],
                (1, 1, 0): [7],
            }
            wo_defer = []  # b0's wo runs during b1's exp-paced slabs,
            #              # where the PE otherwise starves for filler
            for b, g, i in order:
                for tbp in range(b * 4, b * 4 + 2 * g + i + 1):
                    for key in tb_keys(tbp):
                        emit_unit(key)      # prerequisites (idempotent)
                if b == 1 and wo_defer:
                    fillers.extend(wo_defer)
                    wo_defer = []
                for tb in prefeed.get((b, g, i), ()):
                    fillers.extend(tb_keys(tb))
                slab(b, g, i, per_round=2)
                t4 = 2 * g + i
                if b == 0:
                    wo_defer += [("wo", 0, t4, o) for o in range(8)]
                elif (b, g, i) != (1, 0, 0):
                    fillers += [("wo", 1, t4, o) for o in range(8)]
            pump(len(fillers))
            for o in range(8):              # tail: last slab's wo
                u_wo(1, 0, o, drain=True)

    nc.compile()
    return nc


def _host_inputs(x, wq, wk, wv, wo, freqs_cos, freqs_sin):
    """Build the per-core input maps (all host-side transforms are free)."""
    perm = np.concatenate([np.arange(0, HD, 2), np.arange(1, HD, 2)])  # rot-half
    xTf = x.reshape(TOK, D).T.astype(BF16)  # [D, TOK]
    KT = D // 128
    NTB = TOK // 512
    # one contiguous [128, KT*512] row-block per tb:
    #   xT[tb*128 + p, k*512 + c] = xTf[k*128 + p, tb*512 + c]
    xT = np.ascontiguousarray(
        xTf.reshape(KT, 128, NTB, 512).transpose(2, 1, 0, 3)
           .reshape(NTB * 128, KT * 512))

    # signed block-swap P (per 64-dim head): qs_lo = -q_hi, qs_hi = q_lo
    P = np.zeros((HDC, HDC), np.float32)
    for h in range(HPC):
        base = h * HD
        half = HD // 2
        for i in range(half):
            P[base + i, base + half + i] = -1.0
            P[base + half + i, base + i] = 1.0
    PT = np.ascontiguousarray(P.T).astype(BF16)

    # cos/sin expanded to [HDC, TOK]; row j within a head uses freq j%32
    half = HD // 2
    idx = np.concatenate([np.arange(half), np.arange(half)])  # [64]
    cos1 = freqs_cos[:, :].T[idx]  # [64, S]
    sin1 = freqs_sin[:, :].T[idx]
    cosx = np.tile(np.tile(cos1, (HPC, 1)), (1, B)).astype(BF16)  # [128, TOK]
    sinx = np.tile(np.tile(sin1, (HPC, 1)), (1, B)).astype(BF16)

    # additive causal mask for the diagonal tile: 0 where sk<=sq, -1e9 else
    tri = np.where(np.triu(np.ones((128, 128), dtype=bool)), 0.0,
                   -1e9).astype(BF16)
    ident = np.eye(128, dtype=np.float32).astype(BF16)

    def w_relay(wT):
        # [D, 128] -> [128, KT*128]: row p, col k*128+c = wT[k*128+p, c]
        return np.ascontiguousarray(
            wT.reshape(KT, 128, HDC).transpose(1, 0, 2).reshape(128, KT * HDC))

    in_maps = []
    for c in range(NC):
        rows = []
        for h in range(HPC):
            hg = c * HPC + h
            rows.append(hg * HD + perm)
        rows = np.concatenate(rows)
        wq_c = w_relay(wq[rows, :].T.astype(BF16))
        wk_c = w_relay(wk[rows, :].T.astype(BF16))
        vrows = np.arange(c * HDC, (c + 1) * HDC)
        wv_c = w_relay(wv[vrows, :].T.astype(BF16))
        wo_c = np.ascontiguousarray(wo[:, vrows].T).astype(BF16)  # [128, D]
        in_maps.append({
            "xT": xT, "wqT": wq_c, "wkT": wk_c, "wvT": wv_c, "woT": wo_c,
            "PT": PT, "cosx": cosx, "sinx": sinx, "tri": tri,
            "ident": ident,
        })
    return in_maps


def _install_ntff_hook():
    """Provide antenv.axon_hooks (missing in this image) so that
    run_bass_kernel_spmd(trace=True) can capture an NTFF profile via the
    axon PJRT .so — replicates trn_boot._ntff_profile_via_ctypes."""
    import types, ctypes, contextlib, sys as _sys

    if "antenv.axon_hooks" in _sys.modules:
        return
    so_path = "/opt/axon/libaxon_pjrt.so"
    try:
        lib = ctypes.CDLL(so_path)
    except OSError:
        return
    if not hasattr(lib, "axon_start_nrt_profile"):
        return
    lib.axon_start_nrt_profile.argtypes = [ctypes.POINTER(ctypes.c_int64),
                                           ctypes.c_size_t]
    lib.axon_start_nrt_profile.restype = ctypes.c_int64
    lib.axon_stop_nrt_profile.argtypes = [ctypes.c_char_p]
    lib.axon_stop_nrt_profile.restype = ctypes.c_int64

    @contextlib.contextmanager
    def _hook(output_dir, device_ids):
        import jax
        jax.devices()
        if device_ids:
            ids = (ctypes.c_int64 * len(device_ids))(*device_ids)
            rc = lib.axon_start_nrt_profile(ids, len(device_ids))
        else:
            rc = lib.axon_start_nrt_profile(None, 0)
        if rc != 0:
            raise RuntimeError(f"axon_start_nrt_profile rc={rc}")
        try:
            yield
        finally:
            n = lib.axon_stop_nrt_profile(str(output_dir).encode())
            print(f"ntff profile: {n} file(s) -> {output_dir}", file=sys.stderr)

    mod = types.ModuleType("antenv.axon_hooks")
    mod.get_axon_ntff_profile_hook = lambda: _hook
    mod.set_axon_ntff_profile_hook = lambda h: None
    import antenv
    antenv.axon_hooks = mod
    _sys.modules["antenv.axon_hooks"] = mod


def _is_causal_mask(mask):
    ref = np.where(np.tril(np.ones((S, S), dtype=bool)), 0.0, -1e9)
    return mask.shape == (S, S) and np.array_equal(
        mask.astype(np.float32), ref.astype(np.float32))


def kernel(x, wq, wk, wv, wo, freqs_cos, freqs_sin, mask, _want_trace=False):
    x = np.asarray(x, np.float32)
    mask = np.asarray(mask, np.float32)
    if not _is_causal_mask(mask):
        # general fallback (never hit for the reference's causal mask)
        return _numpy_reference(x, wq, wk, wv, wo, freqs_cos, freqs_sin, mask)

    from concourse.bass_utils import run_bass_kernel_spmd

    if _want_trace:
        _install_ntff_hook()
    if "prog" not in _COMPILED:
        _COMPILED["prog"] = _build_program()
    nc = _COMPILED["prog"]

    in_maps = _host_inputs(np.asarray(x, np.float32), np.asarray(wq, np.float32),
                           np.asarray(wk, np.float32), np.asarray(wv, np.float32),
                           np.asarray(wo, np.float32),
                           np.asarray(freqs_cos, np.float32),
                           np.asarray(freqs_sin, np.float32))
    res = run_bass_kernel_spmd(nc, in_maps, core_ids=list(range(NC)),
                               trace=_want_trace)
    total = np.zeros((D, TOK), np.float32)
    for c in range(NC):
        total += res.results[c]["out"].astype(np.float32)
    out = total.T.reshape(B, S, D).astype(np.float32)
    if _want_trace:
        _COMPILED["last_result"] = res
    return out


def _numpy_reference(x, wq, wk, wv, wo, freqs_cos, freqs_sin, mask):
    import math

    def rope(t):
        t2 = t.reshape(*t.shape[:-1], HD // 2, 2)
        x0, x1 = t2[..., 0], t2[..., 1]
        c = freqs_cos[None, :, None, :]
        s = freqs_sin[None, :, None, :]
        r0 = x0 * c - x1 * s
        r1 = x0 * s + x1 * c
        return np.stack([r0, r1], axis=-1).reshape(t.shape)

    b, s, d = x.shape
    q = (x @ wq.T).reshape(b, s, H, HD)
    k = (x @ wk.T).reshape(b, s, H, HD)
    v = (x @ wv.T).reshape(b, s, H, HD)
    q, k = rope(q), rope(k)
    q = q.transpose(0, 2, 1, 3)
    k = k.transpose(0, 2, 1, 3)
    v = v.transpose(0, 2, 1, 3)
    sc = np.einsum("bhqd,bhkd->bhqk", q, k) / math.sqrt(HD) + mask[None, None]
    sc = sc - sc.max(axis=-1, keepdims=True)
    p = np.exp(sc)
    p /= p.sum(axis=-1, keepdims=True)
    o = np.einsum("bhqk,bhkd->bhqd", p, v).transpose(0, 2, 1, 3).reshape(b, s, d)
    return (o @ wo.T).astype(np.float32)


# revision 27
# speedup vs baseline: 1.0307x; 1.0307x over previous
"""Distributed Trainium2 Bass kernel for causal multi-head attention w/ RoPE.

Problem shapes (hardcoded): B=2, S=2048, D=1024, H=16, HD=64.
Sharding: tensor-parallel over heads — each of 8 cores owns 2 heads
(column slice of wq/wk/wv, row slice of wo). Each core emits its partial
x @ woT contribution; the host sums the 8 partials (the "all-reduce").

Per-core math (all matmuls bf16 on the PE, fp32 PSUM accumulation):
  - q,k,v projections from a host-transposed xT [D, B*S].
  - RoPE: rot(q) = q*cos + (P@q)*sin with P a signed block-swap applied
    by one PE matmul; DVE elementwise ops finish the rotation.
  - scores computed transposed: sT[sk, sq] = k_h^T q_h (K=64), causal
    tiles only; exp on ScalarE with the 1/sqrt(HD) scale folded in.
  - causal mask applied ON the PE (identity @ maskT accumulated into the
    scores PSUM).
  - SLAB-MAJOR schedule: attention runs one 512-wide sq slab at a time
    (for kt <= slab end: scores+mask, exp, PV for both heads,
    interleaved).  The slab's two PV accumulators [65,512] live only for
    the slab => PSUM: 4 banks of score ring + 2 banks po + 2 banks for
    filler (projections / rope / v-transpose / wo).
  - PV uses v' = [v | 1] so the softmax denominator falls out of the
    matmul as row 64; normalization = DVE copies + gpsimd
    partition-broadcast + DVE reciprocal + multiply.
  - the PE never idles: between attention rounds the emitter injects
    dependency-free filler (next batch's projections, rope, v', and the
    wo matmuls of ALREADY-normalized slabs).  Each slab's wo becomes
    filler for the next slab, so the drain tail is one slab's wo.
  - input DMA: fine-grained chunks spread over 4 engine queues (per
    queue the transfers serialize at ~60GB/s, so parallelism across
    queues is what sets arrival rate).
"""

import sys

sys.path.insert(0, "/opt/trn_rl_repo")

import numpy as np
import ml_dtypes

B, S, D, H = 2, 2048, 1024, 16
HD = D // H  # 64
NC = 8
HPC = H // NC  # heads per core = 2
HDC = HPC * HD  # head dims per core = 128
TOK = B * S  # 4096
BF16 = ml_dtypes.bfloat16

_COMPILED = {}


def _build_program():
    import concourse.bass as bass
    import concourse.mybir as mybir
    import concourse.bacc as bacc
    from concourse import tile

    f32 = mybir.dt.float32
    bf16 = mybir.dt.bfloat16
    MULT = mybir.AluOpType.mult
    ADD = mybir.AluOpType.add
    EXP = mybir.ActivationFunctionType.Exp
    KT = D // 128  # 8 contraction tiles for projections
    NTB = TOK // 512  # 8 tok blocks of 512
    NVT = TOK // 128  # 32 tok tiles of 128 (v' tiles)
    VW = HD + 1  # 65: v plus ones column

    nc = bacc.Bacc("TRN2", target_bir_lowering=False, debug=False, num_devices=NC)

    # xT relaid on host: row tb*128+p, col k*512+c = xT[k*128+p, tb*512+c]
    xT_d = nc.dram_tensor("xT", [NTB * 128, KT * 512], bf16,
                          kind="ExternalInput").ap()
    # weights relaid: row p, col k*HDC+c = wT[k*128+p, c]
    wqT_d = nc.dram_tensor("wqT", [128, KT * HDC], bf16, kind="ExternalInput").ap()
    wkT_d = nc.dram_tensor("wkT", [128, KT * HDC], bf16, kind="ExternalInput").ap()
    wvT_d = nc.dram_tensor("wvT", [128, KT * HDC], bf16, kind="ExternalInput").ap()
    woT_d = nc.dram_tensor("woT", [HDC, D], bf16, kind="ExternalInput").ap()
    PT_d = nc.dram_tensor("PT", [HDC, HDC], bf16, kind="ExternalInput").ap()
    cos_d = nc.dram_tensor("cosx", [HDC, TOK], bf16, kind="ExternalInput").ap()
    sin_d = nc.dram_tensor("sinx", [HDC, TOK], bf16, kind="ExternalInput").ap()
    tri_d = nc.dram_tensor("tri", [128, 128], bf16, kind="ExternalInput").ap()
    id_d = nc.dram_tensor("ident", [128, 128], bf16, kind="ExternalInput").ap()
    out_d = nc.dram_tensor("out", [D, TOK], bf16, kind="ExternalOutput").ap()

    with tile.TileContext(nc) as tc:
        with (
            tc.tile_pool(name="big", bufs=1) as big,
            tc.tile_pool(name="work", bufs=3) as work,
            tc.tile_pool(name="etp", bufs=8) as etp,
            tc.tile_pool(name="nrm", bufs=3) as nrm,
            tc.tile_pool(name="scps", bufs=4, space="PSUM") as scps,
            tc.tile_pool(name="povp", bufs=2, space="PSUM") as povp,
            tc.tile_pool(name="mscp", bufs=2, space="PSUM") as mscp,
        ):
            # ---- DMA: fine chunks over 4 queues, first-needed first ----
            wq = big.tile([128, KT * HDC], bf16, tag="wq")
            wk = big.tile([128, KT * HDC], bf16, tag="wk")
            wv = big.tile([128, KT * HDC], bf16, tag="wv")
            xT = big.tile([128, KT, TOK], bf16, tag="xT")
            cosx = big.tile([128, TOK], bf16, tag="cosx")
            sinx = big.tile([128, TOK], bf16, tag="sinx")
            tri = big.tile([128, 128], bf16, tag="tri")
            ident = big.tile([128, 128], bf16, tag="ident")
            PT = big.tile([128, 128], bf16, tag="PT")
            wo = big.tile([128, D], bf16, tag="wo")

            def x_dma(tb, k0, k1, eng):
                eng.dma_start(
                    xT[:, k0:k1, tb * 512 : (tb + 1) * 512],
                    xT_d[tb * 128 : (tb + 1) * 128, k0 * 512 : k1 * 512])

            # queue scripts.  Only SP(sync)/Pool(gpsimd)/Act(scalar) can
            # initiate DMA.  A big transfer BLOCKS its issuing queue, so
            # the scalar queue gets only small, early transfers (it must
            # be free for exp once attention starts ~18us in); sync and
            # gpsimd carry the x stream (the 16 HW DMA engines give each
            # queue ~100GB/s).
            nc.sync.dma_start(wq[:], wqT_d[:, :])
            x_dma(0, 0, 2, nc.gpsimd)
            nc.scalar.dma_start(wk[:], wkT_d[:, :])
            x_dma(0, 2, 4, nc.sync)
            x_dma(0, 4, 6, nc.gpsimd)
            nc.scalar.dma_start(wv[:], wvT_d[:, :])
            x_dma(0, 6, 8, nc.sync)
            nc.scalar.dma_start(tri[:], tri_d[:, :])
            nc.scalar.dma_start(ident[:], id_d[:, :])
            nc.scalar.dma_start(PT[:], PT_d[:, :])
            nc.gpsimd.dma_start(cosx[:, 0:1024], cos_d[:, 0:1024])
            nc.sync.dma_start(sinx[:, 0:1024], sin_d[:, 0:1024])
            x_dma(1, 0, 4, nc.gpsimd)
            x_dma(1, 4, 8, nc.sync)
            nc.scalar.dma_start(cosx[:, 1024:2048], cos_d[:, 1024:2048])
            nc.scalar.dma_start(sinx[:, 1024:2048], sin_d[:, 1024:2048])
            x_dma(2, 0, 4, nc.sync)
            x_dma(2, 4, 8, nc.gpsimd)
            nc.scalar.dma_start(wo[:], woT_d[:, :])
            x_dma(3, 0, 4, nc.gpsimd)
            x_dma(3, 4, 8, nc.sync)
            nc.gpsimd.dma_start(cosx[:, 2048:4096], cos_d[:, 2048:4096])
            nc.sync.dma_start(sinx[:, 2048:4096], sin_d[:, 2048:4096])
            for tb in range(4, NTB):
                qa, qb = ((nc.gpsimd, nc.sync),
                          (nc.sync, nc.gpsimd))[tb % 2]
                x_dma(tb, 0, 4, qa)
                x_dma(tb, 4, 8, qb)

            q_sb = big.tile([128, TOK], bf16, tag="q")
            k_sb = big.tile([128, TOK], bf16, tag="k")
            v_sb = big.tile([128, TOK], bf16, tag="v")
            rotq = big.tile([128, TOK], bf16, tag="rotq")
            rotk = big.tile([128, TOK], bf16, tag="rotk")
            vp = big.tile([128, NVT * 2 * VW], bf16, tag="vp")
            oh1 = big.tile([64, S], bf16, tag="oh1")
            outT = {}
            for b in range(B):
                outT[b] = big.tile([128, S], bf16, tag=f"outT{b}",
                                   name=f"outT{b}")

            def vp_head(kt, h):
                base = kt * 2 * VW + h * VW
                return vp[:, base : base + VW]

            # ---------- filler units (each = one mscp psum lifetime) ----
            copy_flip = [0]

            def u_proj(tb, di):
                """one projection dst (q/k/v) for token block tb"""
                dst, w_sb = ((q_sb, wq), (k_sb, wk), (v_sb, wv))[di]
                ps = mscp.tile([128, 512], f32, tag="pj", name=f"pj{tb}{di}")
                for k in range(KT):
                    nc.tensor.matmul(
                        ps[:], w_sb[:, k * HDC : (k + 1) * HDC],
                        xT[:, k, tb * 512 : (tb + 1) * 512],
                        start=(k == 0), stop=(k == KT - 1))
                copy_flip[0] ^= 1
                eng = nc.scalar if copy_flip[0] else nc.vector
                if eng is nc.scalar:
                    nc.scalar.copy(dst[:, tb * 512 : (tb + 1) * 512], ps[:])
                else:
                    nc.vector.tensor_copy(dst[:, tb * 512 : (tb + 1) * 512],
                                          ps[:])

            def u_rope(tb, qi):
                """rope for q (qi=0) or k (qi=1) on token block tb"""
                src, dst = ((q_sb, rotq), (k_sb, rotk))[qi]
                blk = slice(tb * 512, (tb + 1) * 512)
                pss = mscp.tile([128, 512], f32, tag="pj", name=f"pr{tb}{qi}")
                nc.tensor.matmul(pss[:], PT[:], src[:, blk],
                                 start=True, stop=True)
                t1 = work.tile([128, 512], bf16, tag="ropet1")
                nc.vector.tensor_tensor(t1[:], src[:, blk], cosx[:, blk], MULT)
                t2 = work.tile([128, 512], bf16, tag="ropet2")
                nc.vector.tensor_tensor(t2[:], pss[:], sinx[:, blk], MULT)
                nc.vector.tensor_tensor(dst[:, blk], t1[:], t2[:], ADD)

            def u_vp(tb, half):
                """v' tiles for 2 of tb's 4 token tiles (half=0/1)"""
                for kt in (tb * 4 + 2 * half, tb * 4 + 2 * half + 1):
                    pst = mscp.tile([128, 128], bf16, tag="pj",
                                    name=f"pv{kt}")
                    nc.tensor.transpose(pst[:],
                                        v_sb[:, kt * 128 : (kt + 1) * 128],
                                        ident[:])
                    for h in range(HPC):
                        base = kt * 2 * VW + h * VW
                        nc.vector.tensor_copy(vp[:, base : base + HD],
                                              pst[:, h * HD : (h + 1) * HD])
                        nc.gpsimd.memset(vp[:, base + HD : base + VW], 1.0)

            def u_wo(bw, t4, o, drain=False):
                """one wo output tile: matmul + copy + dma"""
                psw = mscp.tile([128, 512], f32, tag="pj",
                                name=f"pw{bw}{t4}{o}")
                nc.tensor.matmul(
                    psw[:], wo[:, o * 128 : (o + 1) * 128],
                    outT[bw][:, t4 * 512 : (t4 + 1) * 512],
                    start=True, stop=True)
                wout = work.tile([128, 512], bf16, tag="wout")
                if o % 2 == 1:
                    nc.scalar.copy(wout[:], psw[:])
                else:
                    nc.vector.tensor_copy(wout[:], psw[:])
                (nc.sync, nc.gpsimd)[o % 2].dma_start(
                    out_d[o * 128 : (o + 1) * 128,
                          bw * S + t4 * 512 : bw * S + (t4 + 1) * 512],
                    wout[:])

            # unit registry: every filler unit is keyed; slabs force-emit
            # their prerequisites (idempotent), extra units pump as filler
            done = set()

            def emit_unit(key):
                if key in done:
                    return
                done.add(key)
                kind = key[0]
                if kind == "proj":
                    u_proj(key[1], key[2])
                elif kind == "rope":
                    u_rope(key[1], key[2])
                elif kind == "vp":
                    u_vp(key[1], key[2])
                elif kind == "wo":
                    u_wo(key[1], key[2], key[3])

            def tb_keys(tb):
                return ([("proj", tb, di) for di in range(3)]
                        + [("rope", tb, 0), ("rope", tb, 1)]
                        + [("vp", tb, 0), ("vp", tb, 1)])

            fillers = []

            def pump(n):
                while n > 0 and fillers:
                    key = fillers.pop(0)
                    if key in done:
                        continue
                    emit_unit(key)
                    n -= 1

            # ---------------- attention slab ---------------------------
            def slab(b, g, i, per_round=1):
                """sq slab [c0, c0+512) of batch b; kt = 0..n_kt-1.

                Per round: S(h1),E(h1),PV(h1,kt-1),S(h0),E(h0),PV(h0,kt-1)
                with `per_round` filler units pumped each round."""
                c0 = 1024 * g + 512 * i
                n_kt = (c0 + 512) // 128
                po = {}
                for h in (1, 0):
                    po[h] = povp.tile([VW, 512], f32, tag="po",
                                      name=f"po{b}{g}{i}{h}")
                sc = {}
                et = {}

                def norm(h):
                    """normalize po[h] -> outT[b] rows h*64..h*64+64.
                    po is read in place (ring 2 = a full slab of slack)."""
                    d0 = nrm.tile([1, 512], f32, tag="d0")
                    nc.vector.tensor_copy(d0[0:1, :], po[h][HD : HD + 1, :])
                    db = nrm.tile([64, 512], f32, tag="db")
                    nc.gpsimd.partition_broadcast(db[:, :], d0[0:1, :])
                    rb_sb = nrm.tile([64, 512], f32, tag="rb")
                    nc.vector.reciprocal_approx_fast(rb_sb[:, :], db[:, :])
                    ocols = slice(c0, c0 + 512)
                    if h == 0:
                        nc.vector.tensor_tensor(outT[b][0:HD, ocols],
                                                po[h][0:HD, :], rb_sb[:, :],
                                                MULT)
                    else:
                        nc.vector.tensor_tensor(oh1[:, ocols], po[h][0:HD, :],
                                                rb_sb[:, :], MULT)
                        nc.sync.dma_start(outT[b][HD : 2 * HD, ocols],
                                          oh1[:, ocols])

                def emit_S(h, kt):
                    hsl = slice(h * HD, (h + 1) * HD)
                    w0 = 128 * kt
                    s0 = max(w0, c0)
                    t = scps.tile([128, 512], f32, tag="sc",
                                  name=f"sc{b}{g}{i}{h}{kt}")
                    sc[(h, kt)] = (t, s0)
                    diag = w0 >= c0
                    nc.tensor.matmul(
                        t[:, s0 - c0 : 512],
                        rotk[hsl, b * S + w0 : b * S + w0 + 128],
                        rotq[hsl, b * S + s0 : b * S + c0 + 512],
                        start=True, stop=not diag,
                    )
                    if diag:
                        nc.tensor.matmul(
                            t[:, w0 - c0 : w0 - c0 + 128],
                            ident[:], tri[:],
                            start=False, stop=True,
                        )

                def emit_E(h, kt):
                    t, s0 = sc.pop((h, kt))
                    tt = etp.tile([128, 512], bf16, tag="expT",
                                  name=f"et{b}{g}{i}{h}{kt}")
                    et[(h, kt)] = (tt, s0)
                    nc.scalar.activation(
                        tt[:, s0 - c0 : 512], t[:, s0 - c0 : 512],
                        EXP, scale=0.125)

                def emit_PV(h, kt):
                    tt, s0 = et.pop((h, kt))
                    nc.tensor.matmul(
                        po[h][:, s0 - c0 : 512],
                        vp_head(b * (NVT // B) + kt, h),
                        tt[:, s0 - c0 : 512],
                        start=(kt == 0), stop=(kt == n_kt - 1),
                    )

                emit_S(1, 0)
                emit_E(1, 0)
                emit_S(0, 0)
                emit_E(0, 0)
                for kt in range(1, n_kt):
                    emit_S(1, kt)
                    emit_E(1, kt)
                    emit_PV(1, kt - 1)
                    pump(per_round - 1)
                    emit_S(0, kt)
                    emit_E(0, kt)
                    emit_PV(0, kt - 1)
                    pump(1)
                emit_PV(1, n_kt - 1)
                norm(1)
                emit_PV(0, n_kt - 1)
                norm(0)

            # ---------------- global schedule ---------------------------
            # slab order: b0 causal order, then b1 with the smallest slab
            # (4 rounds) LAST so the drain tail is only its 8 wo tiles.
            # Before each slab, force-emit its prerequisite tb units
            # (idempotent); queued copies pump as filler inside rounds.
            order = [(0, 0, 0), (0, 0, 1), (0, 1, 0), (0, 1, 1),
                     (1, 0, 1), (1, 1, 0), (1, 1, 1), (1, 0, 0)]
            # seed future-tb units into the filler queue ahead of need
            prefeed = {
                (0, 0, 0): [1],  # during slab 000, start tb1
                (0, 0, 1): [2],
                (0, 1, 0): [3],
                (0, 1, 1): [4, 5],
                (1, 0, 1): [6],
                (1, 1, 0): [7],
            }
            wo_defer = []  # b0's wo runs during b1's exp-paced slabs,
            #              # where the PE otherwise starves for filler
            for b, g, i in order:
                for tbp in range(b * 4, b * 4 + 2 * g + i + 1):
                    for key in tb_keys(tbp):
                        emit_unit(key)      # prerequisites (idempotent)
                if b == 1 and wo_defer:
                    fillers.extend(wo_defer)
                    wo_defer = []
                for tb in prefeed.get((b, g, i), ()):
                    fillers.extend(tb_keys(tb))
                slab(b, g, i, per_round=2)
                t4 = 2 * g + i
                if b == 0:
                    wo_defer += [("wo", 0, t4, o) for o in range(8)]
                elif (b, g, i) != (1, 0, 0):
                    fillers += [("wo", 1, t4, o) for o in range(8)]
            pump(len(fillers))
            for o in range(8):              # tail: last slab's wo
                u_wo(1, 0, o, drain=True)

    nc.compile()
    return nc


def _host_inputs(x, wq, wk, wv, wo, freqs_cos, freqs_sin):
    """Build the per-core input maps (all host-side transforms are free)."""
    perm = np.concatenate([np.arange(0, HD, 2), np.arange(1, HD, 2)])  # rot-half
    xTf = x.reshape(TOK, D).T.astype(BF16)  # [D, TOK]
    KT = D // 128
    NTB = TOK // 512
    # one contiguous [128, KT*512] row-block per tb:
    #   xT[tb*128 + p, k*512 + c] = xTf[k*128 + p, tb*512 + c]
    xT = np.ascontiguousarray(
        xTf.reshape(KT, 128, NTB, 512).transpose(2, 1, 0, 3)
           .reshape(NTB * 128, KT * 512))

    # signed block-swap P (per 64-dim head): qs_lo = -q_hi, qs_hi = q_lo
    P = np.zeros((HDC, HDC), np.float32)
    for h in range(HPC):
        base = h * HD
        half = HD // 2
        for i in range(half):
            P[base + i, base + half + i] = -1.0
            P[base + half + i, base + i] = 1.0
    PT = np.ascontiguousarray(P.T).astype(BF16)

    # cos/sin expanded to [HDC, TOK]; row j within a head uses freq j%32
    half = HD // 2
    idx = np.concatenate([np.arange(half), np.arange(half)])  # [64]
    cos1 = freqs_cos[:, :].T[idx]  # [64, S]
    sin1 = freqs_sin[:, :].T[idx]
    cosx = np.tile(np.tile(cos1, (HPC, 1)), (1, B)).astype(BF16)  # [128, TOK]
    sinx = np.tile(np.tile(sin1, (HPC, 1)), (1, B)).astype(BF16)

    # additive causal mask for the diagonal tile: 0 where sk<=sq, -1e9 else
    tri = np.where(np.triu(np.ones((128, 128), dtype=bool)), 0.0,
                   -1e9).astype(BF16)
    ident = np.eye(128, dtype=np.float32).astype(BF16)

    def w_relay(wT):
        # [D, 128] -> [128, KT*128]: row p, col k*128+c = wT[k*128+p, c]
        return np.ascontiguousarray(
            wT.reshape(KT, 128, HDC).transpose(1, 0, 2).reshape(128, KT * HDC))

    in_maps = []
    for c in range(NC):
        rows = []
        for h in range(HPC):
            hg = c * HPC + h
            rows.append(hg * HD + perm)
        rows = np.concatenate(rows)
        wq_c = w_relay(wq[rows, :].T.astype(BF16))
        wk_c = w_relay(wk[rows, :].T.astype(BF16))
        vrows = np.arange(c * HDC, (c + 1) * HDC)
        wv_c = w_relay(wv[vrows, :].T.astype(BF16))
        wo_c = np.ascontiguousarray(wo[:, vrows].T).astype(BF16)  # [128, D]
        in_maps.append({
            "xT": xT, "wqT": wq_c, "wkT": wk_c, "wvT": wv_c, "woT": wo_c,
            "PT": PT, "cosx": cosx, "sinx": sinx, "tri": tri,
            "ident": ident,
        })
    return in_maps


def _install_ntff_hook():
    """Provide antenv.axon_hooks (missing in this image) so that
    run_bass_kernel_spmd(trace=True) can capture an NTFF profile via the
    axon PJRT .so — replicates trn_boot._ntff_profile_via_ctypes."""
    import types, ctypes, contextlib, sys as _sys

    if "antenv.axon_hooks" in _sys.modules:
        return
    so_path = "/opt/axon/libaxon_pjrt.so"
    try:
        lib = ctypes.CDLL(so_path)
    except OSError:
        return
    if not hasattr(lib, "axon_start_nrt_profile"):
        return
    lib.axon_start_nrt_profile.argtypes = [ctypes.POINTER(ctypes.c_int64),
                                           ctypes.c_size_t]
    lib.axon_start_nrt_profile.restype = ctypes.c_int64
    lib.axon_stop_nrt_profile.argtypes = [ctypes.c_char_p]
    lib.axon_stop_nrt_profile.restype = ctypes.c_int64

    @contextlib.contextmanager
    def _hook(output_dir, device_ids):
        import jax
        jax.devices()
        if device_ids:
            ids = (ctypes.c_int64 * len(device_ids))(*device_ids)
            rc = lib.axon_start_nrt_profile(ids, len(device_ids))
        else:
            rc = lib.axon_start_nrt_profile(None, 0)
        if rc != 0:
            raise RuntimeError(f"axon_start_nrt_profile rc={rc}")
        try:
            yield
        finally:
            n = lib.axon_stop_nrt_profile(str(output_dir).encode())
            print(f"ntff profile: {n} file(s) -> {output_dir}", file=sys.stderr)

    mod = types.ModuleType("antenv.axon_hooks")
    mod.get_axon_ntff_profile_hook = lambda: _hook
    mod.set_axon_ntff_profile_hook = lambda h: None
    import antenv
    antenv.axon_hooks = mod
    _sys.modules["antenv.axon_hooks"] = mod


def _is_causal_mask(mask):
    ref = np.where(np.tril(np.ones((S, S), dtype=bool)), 0.0, -1e9)
    return mask.shape == (S, S) and np.array_equal(
        mask.astype(np.float32), ref.astype(np.float32))


def kernel(x, wq, wk, wv, wo, freqs_cos, freqs_sin, mask, _want_trace=False):
    x = np.asarray(x, np.float32)
    mask = np.asarray(mask, np.float32)
    if not _is_causal_mask(mask):
        # general fallback (never hit for the reference's causal mask)
        return _numpy_reference(x, wq, wk, wv, wo, freqs_cos, freqs_sin, mask)

    from concourse.bass_utils import run_bass_kernel_spmd

    if _want_trace:
        _install_ntff_hook()
    if "prog" not in _COMPILED:
        _COMPILED["prog"] = _build_program()
    nc = _COMPILED["prog"]

    in_maps = _host_inputs(np.asarray(x, np.float32), np.asarray(wq, np.float32),
                           np.asarray(wk, np.float32), np.asarray(wv, np.float32),
                           np.asarray(wo, np.float32),
                           np.asarray(freqs_cos, np.float32),
                           np.asarray(freqs_sin, np.float32))
    res = run_bass_kernel_spmd(nc, in_maps, core_ids=list(range(NC)),
                               trace=_want_trace)
    total = np.zeros((D, TOK), np.float32)
    for c in range(NC):
        total += res.results[c]["out"].astype(np.float32)
    out = total.T.reshape(B, S, D).astype(np.float32)
    if _want_trace:
        _COMPILED["last_result"] = res
    return out


def _numpy_reference(x, wq, wk, wv, wo, freqs_cos, freqs_sin, mask):
    import math

    def rope(t):
        t2 = t.reshape(*t.shape[:-1], HD // 2, 2)
        x0, x1 = t2[..., 0], t2[..., 1]
        c = freqs_cos[None, :, None, :]
        s = freqs_sin[None, :, None, :]
        r0 = x0 * c - x1 * s
        r1 = x0 * s + x1 * c
        return np.stack([r0, r1], axis=-1).reshape(t.shape)

    b, s, d = x.shape
    q = (x @ wq.T).reshape(b, s, H, HD)
    k = (x @ wk.T).reshape(b, s, H, HD)
    v = (x @ wv.T).reshape(b, s, H, HD)
    q, k = rope(q), rope(k)
    q = q.transpose(0, 2, 1, 3)
    k = k.transpose(0, 2, 1, 3)
    v = v.transpose(0, 2, 1, 3)
    sc = np.einsum("bhqd,bhkd->bhqk", q, k) / math.sqrt(HD) + mask[None, None]
    sc = sc - sc.max(axis=-1, keepdims=True)
    p = np.exp(sc)
    p /= p.sum(axis=-1, keepdims=True)
    o = np.einsum("bhqk,bhkd->bhqd", p, v).transpose(0, 2, 1, 3).reshape(b, s, d)
    return (o @ wo.T).astype(np.float32)
